# revision 40
# baseline (speedup 1.0000x reference)
"""Trainium2 Bass kernel for nn_EncoderTransformer_61194694033513.

Data-parallel over batch B=16 across 8 NeuronCores (2 batch elems per core).
Per core, the whole forward runs out of SBUF with activations stored
feature-major HT[e, tok] in fp16 (matmul operands must be 16-bit to stream at
1 column/cycle on the PE; fp16 carries 10 mantissa bits vs bf16's 7, and
squares are pre-scaled by 1/64 to stay in fp16 range). All matmul accumulation
is fp32 in PSUM. Attention is computed flash-style (S^T tiles of
[128 keys x 512 queries], relu, accumulated into O^T) so the [N,N] matrix is
never materialized.

The two batch streams are software-pipelined against each other so the PE
never waits on the LayerNorm scalar/vector chains: per layer the emission
order is  attn(b) [with LN1 stats lagged one 512-token chunk behind],
qkv(1-b) [covering LN1's row math + apply], mlp(b) [with LN2 stats lagged],
then the trailing LN2 stats/apply of stream b are tucked into the start of
stream 1-b's next attention block. LN reductions over the feature (partition)
axis go through the PE with a ones lhsT into [1,512] PSUM rows; per-token row
math runs on partition 0 (rstd via Abs_reciprocal_sqrt on the scalar engine),
and rstd / mean*rstd rows are broadcast over partitions on GpSimd.
"""

import sys

import numpy as np

for _p in (
    "/opt/trn_rl_repo",
    "/opt/pypackages",
    "/root/.axon_site",
    "/root/.axon_site/_ro/trn_rl_repo",
    "/root/.axon_site/_ro/pypackages",
):
    if _p not in sys.path:
        sys.path.append(_p)

import ml_dtypes  # noqa: E402,F401

import concourse.bass as bass  # noqa: E402
import concourse.bacc as bacc  # noqa: E402
import concourse.mybir as mybir  # noqa: E402
from concourse import tile  # noqa: E402
from concourse.bass_utils import run_bass_kernel_spmd  # noqa: E402

B, N, D, E, L = 16, 2048, 128, 256, 3
NCORES = 8
BL = B // NCORES  # batch elems per core
P = 128
EC = E // P  # feature-dim partition chunks (2)
CH = N // 512  # 512-wide token chunks (4)
JT = N // P  # key tiles (16)
EPS = 1e-5
F32 = mybir.dt.float32
F16 = mybir.dt.float16
NPF16 = np.float16
AF = mybir.ActivationFunctionType
OP = mybir.AluOpType

_CACHE = {}


def _build():
    nc = bacc.Bacc("TRN2", target_bir_lowering=False, debug=False, num_devices=NCORES)

    d_xsT = nc.declare_dram_parameter("xsT", [BL, P, N], F16, isOutput=False)
    d_Win = nc.declare_dram_parameter("Win", [D, E], F16, isOutput=False)
    d_W = {
        nm: nc.declare_dram_parameter(nm, [L, E, E], F16, isOutput=False)
        for nm in ("Wq", "Wk", "Wv", "W1", "W2")
    }
    d_WoutT = nc.declare_dram_parameter("WoutT", [E, N], F16, isOutput=False)
    d_colpack = nc.declare_dram_parameter("colpack", [P, 2 + 6 * L * EC], F32, isOutput=False)
    d_bout = nc.declare_dram_parameter("b_out", [1, 1], F32, isOutput=False)
    d_out = nc.declare_dram_parameter("out", [BL, 1], F32, isOutput=True)

    with tile.TileContext(nc) as tc:
        from contextlib import ExitStack

        with ExitStack() as ctx:
            cpool = ctx.enter_context(tc.tile_pool(name="const", bufs=1))
            hpool = ctx.enter_context(tc.tile_pool(name="acts", bufs=1))
            xs_pool = ctx.enter_context(tc.tile_pool(name="xs", bufs=2))
            spool = ctx.enter_context(tc.tile_pool(name="srelu", bufs=4))
            sqpool = ctx.enter_context(tc.tile_pool(name="sqp", bufs=4))
            apool = ctx.enter_context(tc.tile_pool(name="mlpa", bufs=3))
            tpool = ctx.enter_context(tc.tile_pool(name="t1p", bufs=2))
            ropool = ctx.enter_context(tc.tile_pool(name="ro", bufs=2))

            PS = bass.MemorySpace.PSUM
            ps_s = ctx.enter_context(tc.tile_pool(name="ps_s", bufs=2, space=PS))
            ps_o = ctx.enter_context(tc.tile_pool(name="ps_o", bufs=2, space=PS))
            ps_mm = ctx.enter_context(tc.tile_pool(name="ps_mm", bufs=2, space=PS))

            # ---- input DMAs, ordered so the input projection and the first
            # K projection can start while the rest streams in --------------
            xs_tiles = [
                xs_pool.tile([P, N], F16, name=f"xst{b}", tag=f"xst{b}")
                for b in range(BL)
            ]
            nc.sync.dma_start(xs_tiles[0][:], d_xsT[0])
            win_sb = cpool.tile([P, E], F16, name="win", tag="win")
            nc.sync.dma_start(win_sb[:], d_Win[:])
            colpack = cpool.tile([P, 2 + 6 * L * EC], F32, name="colpack", tag="colpack")
            nc.sync.dma_start(colpack[:], d_colpack[:])

            # one DMA per (name, ec) loads all L layers into a [P, L*E] tile
            w_big = {
                nm: [
                    cpool.tile([P, L * E], F16, name=f"{nm}B{ec}", tag=f"{nm}B{ec}")
                    for ec in range(EC)
                ]
                for nm in ("Wq", "Wk", "Wv", "W1", "W2")
            }

            def _w_dma(nm):
                for ec in range(EC):
                    nc.sync.dma_start(
                        w_big[nm][ec][:].rearrange("p (l e) -> p l e", l=L),
                        d_W[nm][:, ec * P : (ec + 1) * P, :].rearrange("l p e -> p l e"),
                    )

            _w_dma("Wk")
            nc.sync.dma_start(xs_tiles[1][:], d_xsT[1])
            _w_dma("Wq")
            _w_dma("Wv")
            w_sb = {
                nm: [
                    [w_big[nm][ec][:, l * E : (l + 1) * E] for ec in range(EC)]
                    for l in range(L)
                ]
                for nm in ("Wq", "Wk", "Wv", "W1", "W2")
            }

            _w_dma("W1")
            _w_dma("W2")
            binp_sb = colpack[:, 0:EC]

            def col_views(base):
                return [
                    colpack[:, 2 + base * L * EC + l * EC : 2 + base * L * EC + (l + 1) * EC]
                    for l in range(L)
                ]

            bm1_sb = col_views(0)
            bm2_sb = col_views(1)
            be1_sb = col_views(2)
            be2_sb = col_views(3)
            g1_sb = col_views(4)
            g2_sb = col_views(5)
            bout_sb = cpool.tile([1, 1], F32, name="bout", tag="bout")
            nc.sync.dma_start(bout_sb[:], d_bout[:])
            wout_sb = []
            for ec in range(EC):
                t = cpool.tile([P, N], F16, name=f"wout{ec}", tag=f"wout{ec}")
                nc.sync.dma_start(t[:], d_WoutT[ec * P : (ec + 1) * P, :])
                wout_sb.append(t)

            ones_kb = cpool.tile([P, 1], F16, name="ones_kb", tag="ones_kb")
            nc.vector.memset(ones_kb[:], 1.0)
            # full-width ones lhsT: the LN stats matmuls write the token sums
            # replicated across all 128 PSUM partitions at the same cycle
            # cost, so no partition broadcast is ever needed.
            ones_bb = cpool.tile([P, P], F16, name="ones_bb", tag="ones_bb")
            nc.vector.memset(ones_bb[:], 1.0)
            eps_col = cpool.tile([P, 1], F32, name="eps_col", tag="eps_col")
            nc.vector.memset(eps_col[:], EPS)

            # LN per-chunk scratch: mu / var (fp32, broadcast across
            # partitions) and the resulting rstd / mean*rstd fp16 tiles that
            # feed the applies (kept until the lagged apply consumes them).
            lnpool = ctx.enter_context(tc.tile_pool(name="lnp", bufs=2))
            rpool = ctx.enter_context(tc.tile_pool(name="rsp", bufs=8))
            ln_rs = {}

            # ---- persistent activations (fp16), one set per batch elem ----
            Hf = [[hpool.tile([P, N], F16, name=f"Hf{b}{ec}", tag=f"Hf{b}{ec}") for ec in range(EC)] for b in range(BL)]
            qT = [[hpool.tile([P, N], F16, name=f"qT{b}{dc}", tag=f"qT{b}{dc}") for dc in range(EC)] for b in range(BL)]
            kT = [[hpool.tile([P, N], F16, name=f"kT{b}{dc}", tag=f"kT{b}{dc}") for dc in range(EC)] for b in range(BL)]
            v_sb = [hpool.tile([P, JT * E], F16, name=f"v{b}", tag=f"v{b}") for b in range(BL)]

            # ================= emit helpers (chunk granular) =================

            def input_proj(b):
                xs_t = xs_tiles[b]
                for ec in range(EC):
                    es = slice(ec * P, (ec + 1) * P)
                    for c in range(CH):
                        cs = slice(c * 512, (c + 1) * 512)
                        ps = ps_mm.tile([P, 512], F32, name="psin", tag="mm")
                        nc.tensor.matmul(ps[:], win_sb[:, es], xs_t[:, cs])
                        nc.vector.tensor_scalar_add(Hf[b][ec][:, cs], ps[:], binp_sb[:, ec : ec + 1])

            def _proj_group(b, l, w_name, dstT, dc, c):
                ds_ = slice(dc * P, (dc + 1) * P)
                cs = slice(c * 512, (c + 1) * 512)
                ps = ps_mm.tile([P, 512], F32, name="psqk", tag="mm")
                for ec in range(EC):
                    nc.tensor.matmul(
                        ps[:],
                        w_sb[w_name][l][ec][:, ds_],
                        Hf[b][ec][:, cs],
                        start=(ec == 0),
                        stop=(ec == EC - 1),
                    )
                if (dc + c) % 2 == 0:
                    nc.scalar.copy(dstT[dc][:, cs], ps[:])
                else:
                    nc.vector.tensor_copy(dstT[dc][:, cs], ps[:])

            def _v_group(b, l, t):
                ps = ps_mm.tile([P, E], F32, name="psv", tag="mm")
                for ec in range(EC):
                    nc.tensor.matmul(
                        ps[:],
                        Hf[b][ec][:, t * P : (t + 1) * P],
                        w_sb["Wv"][l][ec][:],
                        start=(ec == 0),
                        stop=(ec == EC - 1),
                    )
                if t % 2 == 0:
                    nc.scalar.copy(v_sb[b][:, t * E : (t + 1) * E], ps[:])
                else:
                    nc.vector.tensor_copy(v_sb[b][:, t * E : (t + 1) * E], ps[:])

            def qkv_part(b, l, part):
                """part 0: K projection. part 1: Q projection interleaved
                with V (evens the PSUM-evacuation load over the window)."""
                if part == 0:
                    for dc in range(EC):
                        for c in range(CH):
                            _proj_group(b, l, "Wk", kT[b], dc, c)
                else:
                    for dc in range(EC):
                        for c in range(CH):
                            _proj_group(b, l, "Wq", qT[b], dc, c)
                            _v_group(b, l, (dc * CH + c) * 2)
                            _v_group(b, l, (dc * CH + c) * 2 + 1)

            def attn_chunk(b, c):
                cs = slice(c * 512, (c + 1) * 512)
                o_ps = [
                    ps_o.tile([P, 512], F32, name=f"o{oc}", tag="o")
                    for oc in range(EC)
                ]
                for j2 in range(JT // 2):
                    s_ps = ps_s.tile([P, 1024], F32, name="s_ps", tag="s")
                    # one Nf=1024 matmul per (key-pair, d-chunk): rhs is the
                    # same qT 512-chunk for both key tiles via a 3D AP
                    for h in range(2):
                        j = 2 * j2 + h
                        hs = slice(h * 512, (h + 1) * 512)
                        for dc in range(EC):
                            nc.tensor.matmul(
                                s_ps[:, hs],
                                kT[b][dc][:, j * P : (j + 1) * P],
                                qT[b][dc][:, cs],
                                start=(dc == 0),
                                stop=(dc == EC - 1),
                            )
                    sr = spool.tile([P, 1024], F16, name="sr", tag="sr")
                    if j2 in (2, 5, 7):
                        nc.vector.tensor_relu(sr[:], s_ps[:])
                    else:
                        nc.scalar.activation(sr[:], s_ps[:], AF.Relu)
                    for h in range(2):
                        j = 2 * j2 + h
                        hs = slice(h * 512, (h + 1) * 512)
                        for oc in range(EC):
                            nc.tensor.matmul(
                                o_ps[oc][:],
                                v_sb[b][:, j * E + oc * P : j * E + (oc + 1) * P],
                                sr[:, hs],
                                start=(j == 0),
                                stop=(j == JT - 1),
                            )
                for oc in range(EC):
                    nc.vector.tensor_add(Hf[b][oc][:, cs], Hf[b][oc][:, cs], o_ps[oc][:])

            def stats_chunk(b, c):
                """LN stats + row math for one 512-token chunk. The ones
                lhsT is full-width, so the PE writes the sums replicated
                across all 128 partitions and the row math runs 128-wide;
                the resulting rstd / mean*rstd broadcast tiles are stashed
                in ln_rs for the lagged apply_chunk."""
                X = Hf[b]
                cs = slice(c * 512, (c + 1) * 512)
                sqc = []
                for pt in range(EC):
                    sq = sqpool.tile([P, 512], F16, name="sq", tag="sq")
                    nc.scalar.activation(sq[:], X[pt][:, cs], AF.Square, scale=1.0 / 64)
                    sqc.append(sq)
                st_s = ps_mm.tile([P, 512], F32, name="st_s", tag="mm")
                nc.tensor.matmul(st_s[:], ones_bb[:], X[0][:, cs], start=True, stop=False)
                nc.tensor.matmul(st_s[:], ones_bb[:], X[1][:, cs], start=False, stop=True)
                st_q = ps_mm.tile([P, 512], F32, name="st_q", tag="mm")
                nc.tensor.matmul(st_q[:], ones_bb[:], sqc[0][:], start=True, stop=False)
                nc.tensor.matmul(st_q[:], ones_bb[:], sqc[1][:], start=False, stop=True)
                # mu = sum/E; mu^2*(E/4096) via scalar Square;
                # var*(E/4096) = stq - that; rstd = 1/sqrt(var+eps); mu*rstd.
                mu = lnpool.tile([P, 512], F32, name="mu", tag="mu")
                nc.scalar.activation(mu[:], st_s[:], AF.Copy, scale=1.0 / E)
                sq0 = lnpool.tile([P, 512], F32, name="sq0", tag="sq0")
                nc.scalar.activation(
                    sq0[:], mu[:], AF.Square, scale=float(np.sqrt(E) / 64.0)
                )
                nc.vector.scalar_tensor_tensor(
                    sq0[:], sq0[:], -1.0, st_q[:], op0=OP.mult, op1=OP.add
                )
                rstd = rpool.tile([P, 512], F16, name="rstd", tag="rstd")
                nc.scalar.activation(
                    rstd[:], sq0[:], AF.Abs_reciprocal_sqrt,
                    bias=eps_col[:], scale=4096.0 / E,
                )
                mrstd = rpool.tile([P, 512], F16, name="mrstd", tag="mrstd")
                nc.vector.tensor_mul(mrstd[:], mu[:], rstd[:])
                ln_rs[(b, c)] = (rstd, mrstd)

            def apply_chunk(b, c, g_col, be_col):
                """LN apply over one 512-token chunk."""
                X = Hf[b]
                cs = slice(c * 512, (c + 1) * 512)
                rstd, mrstd = ln_rs.pop((b, c))
                for pt in range(EC):
                    t1 = tpool.tile([P, 512], F16, name="t1", tag="t1")
                    nc.vector.tensor_mul(t1[:], X[pt][:, cs], rstd[:])
                    t2 = tpool.tile([P, 512], F16, name="t2", tag="t2")
                    nc.vector.tensor_sub(t2[:], t1[:], mrstd[:])
                    # X = (x*rstd - mean*rstd)*g + be
                    nc.vector.tensor_scalar(
                        X[pt][:, cs], t2[:],
                        g_col[:, pt : pt + 1], be_col[:, pt : pt + 1],
                        op0=OP.mult, op1=OP.add,
                    )

            def mlp_chunk(b, l, c):
                cs = slice(c * 512, (c + 1) * 512)
                a_t = []
                for mc in range(EC):
                    ms = slice(mc * P, (mc + 1) * P)
                    ps = ps_mm.tile([P, 512], F32, name="psa", tag="mm")
                    for ec in range(EC):
                        nc.tensor.matmul(
                            ps[:],
                            w_sb["W1"][l][ec][:, ms],
                            Hf[b][ec][:, cs],
                            start=(ec == 0),
                            stop=(ec == EC - 1),
                        )
                    a = apool.tile([P, 512], F16, name="a", tag="a")
                    nc.scalar.activation(
                        a[:], ps[:], AF.Relu, bias=bm1_sb[l][:, mc : mc + 1]
                    )
                    a_t.append(a)
                for oc in range(EC):
                    os_ = slice(oc * P, (oc + 1) * P)
                    ps = ps_mm.tile([P, 512], F32, name="psm", tag="mm")
                    for mc in range(EC):
                        nc.tensor.matmul(
                            ps[:],
                            w_sb["W2"][l][mc][:, os_],
                            a_t[mc][:],
                            start=(mc == 0),
                            stop=(mc == EC - 1),
                        )
                    nc.vector.scalar_tensor_tensor(
                        Hf[b][oc][:, cs],
                        ps[:],
                        bm2_sb[l][:, oc : oc + 1],
                        Hf[b][oc][:, cs],
                        op0=OP.add,
                        op1=OP.add,
                    )

            def readout_chunk(b, st, c):
                # token-partial row: psum[0, t] accumulates sum_e H[e,t]*W[e,t]
                # over both partition tiles via fp16 ones-reduce matmuls; all
                # eight (chunk, ec) products accumulate into one [1,512] psum.
                cs = slice(c * 512, (c + 1) * 512)
                for ec in range(EC):
                    ros = ropool.tile([P, 512], F16, name="ros", tag="ros")
                    nc.vector.tensor_mul(ros[:], Hf[b][ec][:, cs], wout_sb[ec][:, cs])
                    nc.tensor.matmul(
                        st[:], ones_kb[:], ros[:],
                        start=(c == 0 and ec == 0),
                        stop=(c == CH - 1 and ec == EC - 1),
                    )

            def readout_pair(b, st, c2):
                readout_chunk(b, st, 2 * c2)
                readout_chunk(b, st, 2 * c2 + 1)

            def readout_finish(b, st):
                rsc = ropool.tile([1, 1], F32, name="rsc", tag="rsc")
                nc.vector.reduce_sum(rsc[:], st[:], axis=mybir.AxisListType.X)
                ob = ropool.tile([1, 1], F32, name="ob", tag="ob")
                nc.scalar.activation(ob[:], rsc[:], AF.Identity, bias=bout_sb[:])
                nc.sync.dma_start(d_out[b : b + 1, :], ob[:])

            def readout(b):
                st = ps_mm.tile([1, 512], F32, name="psro", tag="mm")
                readout_pair(b, st, 0)
                readout_pair(b, st, 1)
                readout_finish(b, st)

            # ================= schedule =================
            # Prologue: input projections + stream-0 layer-0 qkv.
            input_proj(0)
            qkv_part(0, 0, 0)
            input_proj(1)
            qkv_part(0, 0, 1)

            pend = None  # (b, l) whose LN2 chunk-3 stats + applies are deferred
            for l in range(L):
                for b in range(BL):
                    nb = 1 - b
                    lnb = l if b == 0 else l + 1
                    # Flush the other stream's trailing LN2 work under this
                    # attention block's PE cover. The stats (chunk 3) go
                    # first so the row-math chain starts early; the DVE
                    # apply ops are deferred one chunk each so they never
                    # sit ahead of this block's relus in the DVE queue.
                    if pend is not None:
                        pb, pl = pend
                        stats_chunk(pb, 3)
                    attn_chunk(b, 0)
                    if pend is not None:
                        apply_chunk(pb, 0, g2_sb[pl], be2_sb[pl])
                        apply_chunk(pb, 1, g2_sb[pl], be2_sb[pl])
                    attn_chunk(b, 1)
                    if pend is not None:
                        apply_chunk(pb, 2, g2_sb[pl], be2_sb[pl])
                        apply_chunk(pb, 3, g2_sb[pl], be2_sb[pl])
                        pend = None
                    # LN1 stats lagged one chunk behind attention.
                    stats_chunk(b, 0)
                    attn_chunk(b, 2)
                    stats_chunk(b, 1)
                    attn_chunk(b, 3)
                    stats_chunk(b, 2)
                    # Other stream's qkv (or stream-0 readout at the end)
                    # covers this stream's LN1 row math + apply; the apply
                    # DVE ops sit behind the qkv evacuation, not the relus.
                    if lnb < L:
                        qkv_part(nb, lnb, 0)
                        apply_chunk(b, 0, g1_sb[l], be1_sb[l])
                        apply_chunk(b, 1, g1_sb[l], be1_sb[l])
                        qkv_part(nb, lnb, 1)
                    else:
                        readout(0)
                        apply_chunk(b, 0, g1_sb[l], be1_sb[l])
                        apply_chunk(b, 1, g1_sb[l], be1_sb[l])
                    apply_chunk(b, 2, g1_sb[l], be1_sb[l])
                    # MLP with LN2 stats lagged one chunk; LN1's chunk-3
                    # stats/apply ride the mlp window (not the congested qkv
                    # window), and LN2's chunk-3 work is deferred into the
                    # next PE block.
                    mlp_chunk(b, l, 0)
                    stats_chunk(b, 3)
                    apply_chunk(b, 3, g1_sb[l], be1_sb[l])
                    mlp_chunk(b, l, 1)
                    stats_chunk(b, 0)
                    mlp_chunk(b, l, 2)
                    stats_chunk(b, 1)
                    mlp_chunk(b, l, 3)
                    stats_chunk(b, 2)
                    pend = (b, l)

            # Epilogue: finish stream 1's last LN2 chunk-by-chunk so the
            # trailing chunk-3 chain is the only serial tail; readout pieces
            # interleave as their chunks are normalized (stream 0's readout
            # sat in the last qkv slot).
            pb, pl = pend
            stats_chunk(pb, 3)
            apply_chunk(pb, 0, g2_sb[pl], be2_sb[pl])
            apply_chunk(pb, 1, g2_sb[pl], be2_sb[pl])
            st1 = ps_mm.tile([1, 512], F32, name="psro", tag="mm")
            readout_pair(pb, st1, 0)
            apply_chunk(pb, 2, g2_sb[pl], be2_sb[pl])
            readout_chunk(pb, st1, 2)
            apply_chunk(pb, 3, g2_sb[pl], be2_sb[pl])
            readout_chunk(pb, st1, 3)
            readout_finish(pb, st1)

    nc.compile()
    return nc


def _prep_inputs(inputs):
    f = lambda x: np.asarray(x, np.float32)
    bf = lambda x: np.ascontiguousarray(np.asarray(x, np.float32).astype(NPF16))
    xs = f(inputs["xs"])
    xsT = np.ascontiguousarray(xs.transpose(0, 2, 1)).astype(NPF16)  # [B, D, N]
    WoutT = np.ascontiguousarray(f(inputs["Wout"]).reshape(N, E).T).astype(NPF16)  # [E, N]

    def cols(v, per_l):
        v = f(v)
        if per_l:
            return np.ascontiguousarray(v.reshape(L, EC, P).transpose(0, 2, 1))
        return np.ascontiguousarray(v.reshape(EC, P).T)

    common = {
        "Win": bf(inputs["Win"]),
        "Wq": bf(inputs["Wq"]),
        "Wk": bf(inputs["Wk"]),
        "Wv": bf(inputs["Wv"]),
        "W1": bf(inputs["W1"]),
        "W2": bf(inputs["W2"]),
        "WoutT": WoutT,
        "colpack": np.concatenate(
            [cols(inputs["b_in"], False)]
            + [
                cols(inputs[k], True).transpose(1, 0, 2).reshape(P, L * EC)
                for k in ("bm1", "bm2", "be1", "be2", "g1", "g2")
            ],
            axis=1,
        ),
        "b_out": f(inputs["b_out"]).reshape(1, 1),
    }
    in_maps = []
    for c in range(NCORES):
        m = dict(common)
        m["xsT"] = np.ascontiguousarray(xsT[c * BL : (c + 1) * BL])
        in_maps.append(m)
    return in_maps


def get_program():
    if "nc" not in _CACHE:
        _CACHE["nc"] = _build()
    return _CACHE["nc"]


def kernel(**inputs) -> np.ndarray:
    nc = get_program()
    in_maps = _prep_inputs(inputs)
    res = run_bass_kernel_spmd(nc, in_maps, list(range(NCORES)))
    out = np.concatenate([res.results[c]["out"] for c in range(NCORES)], axis=0)
    return out.astype(np.float32)


# revision 41
# speedup vs baseline: 1.0239x; 1.0239x over previous
"""Trainium2 Bass kernel for nn_EncoderTransformer_61194694033513.

Data-parallel over batch B=16 across 8 NeuronCores (2 batch elems per core).
Per core, the whole forward runs out of SBUF with activations stored
feature-major HT[e, tok] in fp16 (matmul operands must be 16-bit to stream at
1 column/cycle on the PE; fp16 carries 10 mantissa bits vs bf16's 7, and
squares are pre-scaled by 1/64 to stay in fp16 range). All matmul accumulation
is fp32 in PSUM. Attention is computed flash-style (S^T tiles of
[128 keys x 512 queries], relu, accumulated into O^T) so the [N,N] matrix is
never materialized.

The two batch streams are software-pipelined against each other so the PE
never waits on the LayerNorm scalar/vector chains: per layer the emission
order is  attn(b) [with LN1 stats lagged one 512-token chunk behind],
qkv(1-b) [covering LN1's row math + apply], mlp(b) [with LN2 stats lagged],
then the trailing LN2 stats/apply of stream b are tucked into the start of
stream 1-b's next attention block. LN reductions over the feature (partition)
axis go through the PE with a ones lhsT into [1,512] PSUM rows; per-token row
math runs on partition 0 (rstd via Abs_reciprocal_sqrt on the scalar engine),
and rstd / mean*rstd rows are broadcast over partitions on GpSimd.
"""

import sys

import numpy as np

for _p in (
    "/opt/trn_rl_repo",
    "/opt/pypackages",
    "/root/.axon_site",
    "/root/.axon_site/_ro/trn_rl_repo",
    "/root/.axon_site/_ro/pypackages",
):
    if _p not in sys.path:
        sys.path.append(_p)

import ml_dtypes  # noqa: E402,F401

import concourse.bass as bass  # noqa: E402
import concourse.bacc as bacc  # noqa: E402
import concourse.mybir as mybir  # noqa: E402
from concourse import tile  # noqa: E402
from concourse.bass_utils import run_bass_kernel_spmd  # noqa: E402

B, N, D, E, L = 16, 2048, 128, 256, 3
NCORES = 8
BL = B // NCORES  # batch elems per core
P = 128
EC = E // P  # feature-dim partition chunks (2)
CH = N // 512  # 512-wide token chunks (4)
JT = N // P  # key tiles (16)
EPS = 1e-5
F32 = mybir.dt.float32
F16 = mybir.dt.float16
NPF16 = np.float16
AF = mybir.ActivationFunctionType
OP = mybir.AluOpType

_CACHE = {}


def _build():
    nc = bacc.Bacc("TRN2", target_bir_lowering=False, debug=False, num_devices=NCORES)

    d_xsT = nc.declare_dram_parameter("xsT", [BL, P, N], F16, isOutput=False)
    d_Win = nc.declare_dram_parameter("Win", [D, E], F16, isOutput=False)
    d_W = {
        nm: nc.declare_dram_parameter(nm, [L, E, E], F16, isOutput=False)
        for nm in ("Wq", "Wk", "Wv", "W1", "W2")
    }
    d_WoutT = nc.declare_dram_parameter("WoutT", [E, N], F16, isOutput=False)
    d_colpack = nc.declare_dram_parameter("colpack", [P, 2 + 6 * L * EC], F32, isOutput=False)
    d_bout = nc.declare_dram_parameter("b_out", [1, 1], F32, isOutput=False)
    d_out = nc.declare_dram_parameter("out", [BL, 1], F32, isOutput=True)

    with tile.TileContext(nc) as tc:
        from contextlib import ExitStack

        with ExitStack() as ctx:
            cpool = ctx.enter_context(tc.tile_pool(name="const", bufs=1))
            hpool = ctx.enter_context(tc.tile_pool(name="acts", bufs=1))
            xs_pool = ctx.enter_context(tc.tile_pool(name="xs", bufs=2))
            spool = ctx.enter_context(tc.tile_pool(name="srelu", bufs=4))
            sqpool = ctx.enter_context(tc.tile_pool(name="sqp", bufs=4))
            apool = ctx.enter_context(tc.tile_pool(name="mlpa", bufs=3))
            tpool = ctx.enter_context(tc.tile_pool(name="t1p", bufs=2))
            ropool = ctx.enter_context(tc.tile_pool(name="ro", bufs=2))

            PS = bass.MemorySpace.PSUM
            ps_s = ctx.enter_context(tc.tile_pool(name="ps_s", bufs=2, space=PS))
            ps_o = ctx.enter_context(tc.tile_pool(name="ps_o", bufs=2, space=PS))
            ps_mm = ctx.enter_context(tc.tile_pool(name="ps_mm", bufs=2, space=PS))

            # ---- input DMAs, ordered so the input projection and the first
            # K projection can start while the rest streams in --------------
            xs_tiles = [
                xs_pool.tile([P, N], F16, name=f"xst{b}", tag=f"xst{b}")
                for b in range(BL)
            ]
            nc.sync.dma_start(xs_tiles[0][:], d_xsT[0])
            win_sb = cpool.tile([P, E], F16, name="win", tag="win")
            nc.sync.dma_start(win_sb[:], d_Win[:])
            colpack = cpool.tile([P, 2 + 6 * L * EC], F32, name="colpack", tag="colpack")
            nc.sync.dma_start(colpack[:], d_colpack[:])

            # one DMA per (name, ec) loads all L layers into a [P, L*E] tile
            w_big = {
                nm: [
                    cpool.tile([P, L * E], F16, name=f"{nm}B{ec}", tag=f"{nm}B{ec}")
                    for ec in range(EC)
                ]
                for nm in ("Wq", "Wk", "Wv", "W1", "W2")
            }

            def _w_dma(nm):
                for ec in range(EC):
                    nc.sync.dma_start(
                        w_big[nm][ec][:].rearrange("p (l e) -> p l e", l=L),
                        d_W[nm][:, ec * P : (ec + 1) * P, :].rearrange("l p e -> p l e"),
                    )

            _w_dma("Wk")
            nc.sync.dma_start(xs_tiles[1][:], d_xsT[1])
            _w_dma("Wq")
            _w_dma("Wv")
            w_sb = {
                nm: [
                    [w_big[nm][ec][:, l * E : (l + 1) * E] for ec in range(EC)]
                    for l in range(L)
                ]
                for nm in ("Wq", "Wk", "Wv", "W1", "W2")
            }

            _w_dma("W1")
            _w_dma("W2")
            binp_sb = colpack[:, 0:EC]

            def col_views(base):
                return [
                    colpack[:, 2 + base * L * EC + l * EC : 2 + base * L * EC + (l + 1) * EC]
                    for l in range(L)
                ]

            bm1_sb = col_views(0)
            bm2_sb = col_views(1)
            be1_sb = col_views(2)
            be2_sb = col_views(3)
            g1_sb = col_views(4)
            g2_sb = col_views(5)
            bout_sb = cpool.tile([1, 1], F32, name="bout", tag="bout")
            nc.sync.dma_start(bout_sb[:], d_bout[:])
            wout_sb = []
            for ec in range(EC):
                t = cpool.tile([P, N], F16, name=f"wout{ec}", tag=f"wout{ec}")
                nc.sync.dma_start(t[:], d_WoutT[ec * P : (ec + 1) * P, :])
                wout_sb.append(t)

            ones_kb = cpool.tile([P, 1], F16, name="ones_kb", tag="ones_kb")
            nc.vector.memset(ones_kb[:], 1.0)
            # full-width ones lhsT: the LN stats matmuls write the token sums
            # replicated across all 128 PSUM partitions at the same cycle
            # cost, so no partition broadcast is ever needed.
            ones_bb = cpool.tile([P, P], F16, name="ones_bb", tag="ones_bb")
            nc.vector.memset(ones_bb[:], 1.0)
            eps_col = cpool.tile([P, 1], F32, name="eps_col", tag="eps_col")
            nc.vector.memset(eps_col[:], EPS)

            # LN per-chunk scratch: mu / var (fp32, broadcast across
            # partitions) and the resulting rstd / mean*rstd fp16 tiles that
            # feed the applies (kept until the lagged apply consumes them).
            lnpool = ctx.enter_context(tc.tile_pool(name="lnp", bufs=2))
            rpool = ctx.enter_context(tc.tile_pool(name="rsp", bufs=8))
            ln_rs = {}

            # ---- persistent activations (fp16), one set per batch elem ----
            Hf = [[hpool.tile([P, N], F16, name=f"Hf{b}{ec}", tag=f"Hf{b}{ec}") for ec in range(EC)] for b in range(BL)]
            qT = [[hpool.tile([P, N], F16, name=f"qT{b}{dc}", tag=f"qT{b}{dc}") for dc in range(EC)] for b in range(BL)]
            kT = [[hpool.tile([P, N], F16, name=f"kT{b}{dc}", tag=f"kT{b}{dc}") for dc in range(EC)] for b in range(BL)]
            v_sb = [hpool.tile([P, JT * E], F16, name=f"v{b}", tag=f"v{b}") for b in range(BL)]

            # ================= emit helpers (chunk granular) =================

            def input_proj(b):
                xs_t = xs_tiles[b]
                for ec in range(EC):
                    es = slice(ec * P, (ec + 1) * P)
                    for c in range(CH):
                        cs = slice(c * 512, (c + 1) * 512)
                        ps = ps_mm.tile([P, 512], F32, name="psin", tag="mm")
                        nc.tensor.matmul(ps[:], win_sb[:, es], xs_t[:, cs])
                        nc.vector.tensor_scalar_add(Hf[b][ec][:, cs], ps[:], binp_sb[:, ec : ec + 1])

            def _proj_group(b, l, w_name, dstT, dc, c):
                ds_ = slice(dc * P, (dc + 1) * P)
                cs = slice(c * 512, (c + 1) * 512)
                ps = ps_mm.tile([P, 512], F32, name="psqk", tag="mm")
                for ec in range(EC):
                    nc.tensor.matmul(
                        ps[:],
                        w_sb[w_name][l][ec][:, ds_],
                        Hf[b][ec][:, cs],
                        start=(ec == 0),
                        stop=(ec == EC - 1),
                    )
                if (dc + c) % 2 == 0:
                    nc.scalar.copy(dstT[dc][:, cs], ps[:])
                else:
                    nc.vector.tensor_copy(dstT[dc][:, cs], ps[:])

            def _v_group(b, l, t):
                ps = ps_mm.tile([P, E], F32, name="psv", tag="mm")
                for ec in range(EC):
                    nc.tensor.matmul(
                        ps[:],
                        Hf[b][ec][:, t * P : (t + 1) * P],
                        w_sb["Wv"][l][ec][:],
                        start=(ec == 0),
                        stop=(ec == EC - 1),
                    )
                if t % 2 == 0:
                    nc.scalar.copy(v_sb[b][:, t * E : (t + 1) * E], ps[:])
                else:
                    nc.vector.tensor_copy(v_sb[b][:, t * E : (t + 1) * E], ps[:])

            def qkv_part(b, l, part):
                """part 0: K projection. part 1: Q projection, then V."""
                if part == 0:
                    for dc in range(EC):
                        for c in range(CH):
                            _proj_group(b, l, "Wk", kT[b], dc, c)
                else:
                    for dc in range(EC):
                        for c in range(CH):
                            _proj_group(b, l, "Wq", qT[b], dc, c)
                    for t in range(JT):
                        _v_group(b, l, t)

            def attn_chunk(b, c):
                cs = slice(c * 512, (c + 1) * 512)
                o_ps = [
                    ps_o.tile([P, 512], F32, name=f"o{oc}", tag="o")
                    for oc in range(EC)
                ]
                for j2 in range(JT // 2):
                    s_ps = ps_s.tile([P, 1024], F32, name="s_ps", tag="s")
                    # one Nf=1024 matmul per (key-pair, d-chunk): rhs is the
                    # same qT 512-chunk for both key tiles via a 3D AP
                    for h in range(2):
                        j = 2 * j2 + h
                        hs = slice(h * 512, (h + 1) * 512)
                        for dc in range(EC):
                            nc.tensor.matmul(
                                s_ps[:, hs],
                                kT[b][dc][:, j * P : (j + 1) * P],
                                qT[b][dc][:, cs],
                                start=(dc == 0),
                                stop=(dc == EC - 1),
                            )
                    sr = spool.tile([P, 1024], F16, name="sr", tag="sr")
                    if j2 in (2, 5, 7):
                        nc.vector.tensor_relu(sr[:], s_ps[:])
                    else:
                        nc.scalar.activation(sr[:], s_ps[:], AF.Relu)
                    for h in range(2):
                        j = 2 * j2 + h
                        hs = slice(h * 512, (h + 1) * 512)
                        for oc in range(EC):
                            nc.tensor.matmul(
                                o_ps[oc][:],
                                v_sb[b][:, j * E + oc * P : j * E + (oc + 1) * P],
                                sr[:, hs],
                                start=(j == 0),
                                stop=(j == JT - 1),
                            )
                for oc in range(EC):
                    nc.vector.tensor_add(Hf[b][oc][:, cs], Hf[b][oc][:, cs], o_ps[oc][:])

            def stats_chunk(b, c):
                """LN stats + row math for one 512-token chunk. The ones
                lhsT is full-width, so the PE writes the sums replicated
                across all 128 partitions and the row math runs 128-wide;
                the resulting rstd / mean*rstd broadcast tiles are stashed
                in ln_rs for the lagged apply_chunk."""
                X = Hf[b]
                cs = slice(c * 512, (c + 1) * 512)
                sqc = []
                for pt in range(EC):
                    sq = sqpool.tile([P, 512], F16, name="sq", tag="sq")
                    nc.scalar.activation(sq[:], X[pt][:, cs], AF.Square, scale=1.0 / 64)
                    sqc.append(sq)
                st_s = ps_mm.tile([P, 512], F32, name="st_s", tag="mm")
                nc.tensor.matmul(st_s[:], ones_bb[:], X[0][:, cs], start=True, stop=False)
                nc.tensor.matmul(st_s[:], ones_bb[:], X[1][:, cs], start=False, stop=True)
                st_q = ps_mm.tile([P, 512], F32, name="st_q", tag="mm")
                nc.tensor.matmul(st_q[:], ones_bb[:], sqc[0][:], start=True, stop=False)
                nc.tensor.matmul(st_q[:], ones_bb[:], sqc[1][:], start=False, stop=True)
                # mu = sum/E; mu^2*(E/4096) via scalar Square;
                # var*(E/4096) = stq - that; rstd = 1/sqrt(var+eps); mu*rstd.
                mu = lnpool.tile([P, 512], F32, name="mu", tag="mu")
                nc.scalar.activation(mu[:], st_s[:], AF.Copy, scale=1.0 / E)
                sq0 = lnpool.tile([P, 512], F32, name="sq0", tag="sq0")
                nc.scalar.activation(
                    sq0[:], mu[:], AF.Square, scale=float(np.sqrt(E) / 64.0)
                )
                nc.vector.scalar_tensor_tensor(
                    sq0[:], sq0[:], -1.0, st_q[:], op0=OP.mult, op1=OP.add
                )
                rstd = rpool.tile([P, 512], F16, name="rstd", tag="rstd")
                nc.scalar.activation(
                    rstd[:], sq0[:], AF.Abs_reciprocal_sqrt,
                    bias=eps_col[:], scale=4096.0 / E,
                )
                mrstd = rpool.tile([P, 512], F16, name="mrstd", tag="mrstd")
                nc.vector.tensor_mul(mrstd[:], mu[:], rstd[:])
                ln_rs[(b, c)] = (rstd, mrstd)

            def apply_chunk(b, c, g_col, be_col):
                """LN apply over one 512-token chunk."""
                X = Hf[b]
                cs = slice(c * 512, (c + 1) * 512)
                rstd, mrstd = ln_rs.pop((b, c))
                for pt in range(EC):
                    t1 = tpool.tile([P, 512], F16, name="t1", tag="t1")
                    nc.vector.tensor_mul(t1[:], X[pt][:, cs], rstd[:])
                    t2 = tpool.tile([P, 512], F16, name="t2", tag="t2")
                    nc.vector.tensor_sub(t2[:], t1[:], mrstd[:])
                    # X = (x*rstd - mean*rstd)*g + be
                    nc.vector.tensor_scalar(
                        X[pt][:, cs], t2[:],
                        g_col[:, pt : pt + 1], be_col[:, pt : pt + 1],
                        op0=OP.mult, op1=OP.add,
                    )

            def mlp_chunk(b, l, c):
                cs = slice(c * 512, (c + 1) * 512)
                a_t = []
                for mc in range(EC):
                    ms = slice(mc * P, (mc + 1) * P)
                    ps = ps_mm.tile([P, 512], F32, name="psa", tag="mm")
                    for ec in range(EC):
                        nc.tensor.matmul(
                            ps[:],
                            w_sb["W1"][l][ec][:, ms],
                            Hf[b][ec][:, cs],
                            start=(ec == 0),
                            stop=(ec == EC - 1),
                        )
                    a = apool.tile([P, 512], F16, name="a", tag="a")
                    nc.scalar.activation(
                        a[:], ps[:], AF.Relu, bias=bm1_sb[l][:, mc : mc + 1]
                    )
                    a_t.append(a)
                for oc in range(EC):
                    os_ = slice(oc * P, (oc + 1) * P)
                    ps = ps_mm.tile([P, 512], F32, name="psm", tag="mm")
                    for mc in range(EC):
                        nc.tensor.matmul(
                            ps[:],
                            w_sb["W2"][l][mc][:, os_],
                            a_t[mc][:],
                            start=(mc == 0),
                            stop=(mc == EC - 1),
                        )
                    nc.vector.scalar_tensor_tensor(
                        Hf[b][oc][:, cs],
                        ps[:],
                        bm2_sb[l][:, oc : oc + 1],
                        Hf[b][oc][:, cs],
                        op0=OP.add,
                        op1=OP.add,
                    )

            def readout_chunk(b, st, c):
                # token-partial row: psum[0, t] accumulates sum_e H[e,t]*W[e,t]
                # over both partition tiles via fp16 ones-reduce matmuls; all
                # eight (chunk, ec) products accumulate into one [1,512] psum.
                cs = slice(c * 512, (c + 1) * 512)
                for ec in range(EC):
                    ros = ropool.tile([P, 512], F16, name="ros", tag="ros")
                    nc.vector.tensor_mul(ros[:], Hf[b][ec][:, cs], wout_sb[ec][:, cs])
                    nc.tensor.matmul(
                        st[:], ones_kb[:], ros[:],
                        start=(c == 0 and ec == 0),
                        stop=(c == CH - 1 and ec == EC - 1),
                    )

            def readout_pair(b, st, c2):
                readout_chunk(b, st, 2 * c2)
                readout_chunk(b, st, 2 * c2 + 1)

            def readout_finish(b, st):
                rsc = ropool.tile([1, 1], F32, name="rsc", tag="rsc")
                nc.vector.reduce_sum(rsc[:], st[:], axis=mybir.AxisListType.X)
                ob = ropool.tile([1, 1], F32, name="ob", tag="ob")
                nc.scalar.activation(ob[:], rsc[:], AF.Identity, bias=bout_sb[:])
                nc.sync.dma_start(d_out[b : b + 1, :], ob[:])

            def readout(b):
                st = ps_mm.tile([1, 512], F32, name="psro", tag="mm")
                readout_pair(b, st, 0)
                readout_pair(b, st, 1)
                readout_finish(b, st)

            # ================= schedule =================
            # Prologue: input projections + stream-0 layer-0 qkv.
            input_proj(0)
            qkv_part(0, 0, 0)
            input_proj(1)
            qkv_part(0, 0, 1)

            pend = None  # (b, l) whose LN2 chunk-3 stats + applies are deferred
            for l in range(L):
                for b in range(BL):
                    nb = 1 - b
                    lnb = l if b == 0 else l + 1
                    # Flush the other stream's trailing LN2 work under this
                    # attention block's PE cover. The stats (chunk 3) go
                    # first so the row-math chain starts early; the DVE
                    # apply ops are deferred one chunk each so they never
                    # sit ahead of this block's relus in the DVE queue.
                    if pend is not None:
                        pb, pl = pend
                        stats_chunk(pb, 3)
                    attn_chunk(b, 0)
                    if pend is not None:
                        apply_chunk(pb, 0, g2_sb[pl], be2_sb[pl])
                        apply_chunk(pb, 1, g2_sb[pl], be2_sb[pl])
                    attn_chunk(b, 1)
                    if pend is not None:
                        apply_chunk(pb, 2, g2_sb[pl], be2_sb[pl])
                        apply_chunk(pb, 3, g2_sb[pl], be2_sb[pl])
                        pend = None
                    # LN1 stats lagged one chunk behind attention.
                    stats_chunk(b, 0)
                    attn_chunk(b, 2)
                    stats_chunk(b, 1)
                    attn_chunk(b, 3)
                    stats_chunk(b, 2)
                    # Other stream's qkv (or stream-0 readout at the end)
                    # covers this stream's LN1 row math + apply; the apply
                    # DVE ops sit behind the qkv evacuation, not the relus.
                    if lnb < L:
                        qkv_part(nb, lnb, 0)
                        apply_chunk(b, 0, g1_sb[l], be1_sb[l])
                        apply_chunk(b, 1, g1_sb[l], be1_sb[l])
                        qkv_part(nb, lnb, 1)
                    else:
                        readout(0)
                        apply_chunk(b, 0, g1_sb[l], be1_sb[l])
                        apply_chunk(b, 1, g1_sb[l], be1_sb[l])
                    apply_chunk(b, 2, g1_sb[l], be1_sb[l])
                    # MLP with LN2 stats lagged one chunk; LN1's chunk-3
                    # stats/apply ride the mlp window (not the congested qkv
                    # window), and LN2's chunk-3 work is deferred into the
                    # next PE block.
                    mlp_chunk(b, l, 0)
                    stats_chunk(b, 3)
                    apply_chunk(b, 3, g1_sb[l], be1_sb[l])
                    mlp_chunk(b, l, 1)
                    stats_chunk(b, 0)
                    mlp_chunk(b, l, 2)
                    stats_chunk(b, 1)
                    mlp_chunk(b, l, 3)
                    stats_chunk(b, 2)
                    pend = (b, l)

            # Epilogue: finish stream 1's last LN2 chunk-by-chunk so the
            # trailing chunk-3 chain is the only serial tail; readout pieces
            # interleave as their chunks are normalized (stream 0's readout
            # sat in the last qkv slot).
            pb, pl = pend
            stats_chunk(pb, 3)
            apply_chunk(pb, 0, g2_sb[pl], be2_sb[pl])
            apply_chunk(pb, 1, g2_sb[pl], be2_sb[pl])
            st1 = ps_mm.tile([1, 512], F32, name="psro", tag="mm")
            readout_pair(pb, st1, 0)
            apply_chunk(pb, 2, g2_sb[pl], be2_sb[pl])
            readout_chunk(pb, st1, 2)
            apply_chunk(pb, 3, g2_sb[pl], be2_sb[pl])
            readout_chunk(pb, st1, 3)
            readout_finish(pb, st1)

    nc.compile()
    return nc


def _prep_inputs(inputs):
    f = lambda x: np.asarray(x, np.float32)
    bf = lambda x: np.ascontiguousarray(np.asarray(x, np.float32).astype(NPF16))
    xs = f(inputs["xs"])
    xsT = np.ascontiguousarray(xs.transpose(0, 2, 1)).astype(NPF16)  # [B, D, N]
    WoutT = np.ascontiguousarray(f(inputs["Wout"]).reshape(N, E).T).astype(NPF16)  # [E, N]

    def cols(v, per_l):
        v = f(v)
        if per_l:
            return np.ascontiguousarray(v.reshape(L, EC, P).transpose(0, 2, 1))
        return np.ascontiguousarray(v.reshape(EC, P).T)

    common = {
        "Win": bf(inputs["Win"]),
        "Wq": bf(inputs["Wq"]),
        "Wk": bf(inputs["Wk"]),
        "Wv": bf(inputs["Wv"]),
        "W1": bf(inputs["W1"]),
        "W2": bf(inputs["W2"]),
        "WoutT": WoutT,
        "colpack": np.concatenate(
            [cols(inputs["b_in"], False)]
            + [
                cols(inputs[k], True).transpose(1, 0, 2).reshape(P, L * EC)
                for k in ("bm1", "bm2", "be1", "be2", "g1", "g2")
            ],
            axis=1,
        ),
        "b_out": f(inputs["b_out"]).reshape(1, 1),
    }
    in_maps = []
    for c in range(NCORES):
        m = dict(common)
        m["xsT"] = np.ascontiguousarray(xsT[c * BL : (c + 1) * BL])
        in_maps.append(m)
    return in_maps


def get_program():
    if "nc" not in _CACHE:
        _CACHE["nc"] = _build()
    return _CACHE["nc"]


def kernel(**inputs) -> np.ndarray:
    nc = get_program()
    in_maps = _prep_inputs(inputs)
    res = run_bass_kernel_spmd(nc, in_maps, list(range(NCORES)))
    out = np.concatenate([res.results[c]["out"] for c in range(NCORES)], axis=0)
    return out.astype(np.float32)
